# revision 19
# baseline (speedup 1.0000x reference)
"""BatchHardTripletLoss on 8 Trainium2 NeuronCores.

Strategy (data-parallel over anchor rows, samples pre-sorted by label):
  - host sorts samples by label (loss is permutation-invariant); core c owns
    anchor rows [c*512, (c+1)*512).
  - sorting clusters each core's 512 rows into <=126 distinct label classes,
    so the same-label penalty matrix PEN * 1[label_m == label_n] restricted to
    a core's rows is RANK <= 126: PEN * onehot_m . onehot_n over class dims.
    It is injected by ONE extra K=128 matmul per output tile (values 128*128 =
    PEN exactly in fp16). Two spare class dims carry the column terms
    (|e_n|^2 - 2*eps*sum(e_n), split into fp16 hi+lo rows against ones).
  - per (m,n) tile: pen/colterm matmul (start=True) + 4 gram matmuls
    accumulate  w = colterm[n] + PEN*same - 2 e_m.e_n  directly in PSUM.
  - DVE mines row-max (hardest positive + PEN) and row-min (hardest negative)
    straight out of PSUM; row-constant terms are applied on the host.
  - host: subtract PEN, add row terms, sqrt, validity via label bincount, mean.
"""

import numpy as np

import concourse.bacc as bacc
import concourse.mybir as mybir
from concourse.bass_utils import run_bass_kernel_spmd
from concourse.tile import TileContext

B = 4096          # batch (anchors)
D = 512           # embedding dim
N_CORES = 8
ROWS = B // N_CORES      # 512 anchor rows per core
P = 128                  # partitions
MT = ROWS // P           # 4 m-tiles per core
NW = 512                 # psum bank width (fp32)
GW = 2048                # column group width (4 banks)
NG = B // GW             # 2 column groups
KT = D // P              # 4 contraction tiles

PEN = 16384.0            # same-label penalty; must exceed max d2 (~2.7k here)
NCLASS_MAX = 126         # class dims per core (2 reserved for colterm hi/lo)
MARGIN = 0.5
EPS = 1e-6

_nc_cache = {}


def _build(reps=1):
    nc = bacc.Bacc("TRN2", target_bir_lowering=False)
    fp16 = mybir.dt.float16
    f32 = mybir.dt.float32

    et = nc.dram_tensor("et", [D, B], fp16, kind="ExternalInput")
    eblk = nc.dram_tensor("eblk", [D, ROWS], fp16, kind="ExternalInput")
    penl = nc.dram_tensor("penl", [P, ROWS], fp16, kind="ExternalInput")
    penr = nc.dram_tensor("penr", [P, B], fp16, kind="ExternalInput")
    outd = nc.dram_tensor("out", [reps, 2, MT, P], f32, kind="ExternalOutput")

    with TileContext(nc) as tc:
        with (
            tc.tile_pool(name="etp", bufs=1) as etp,
            tc.tile_pool(name="ebp", bufs=1) as ebp,
            tc.tile_pool(name="accp", bufs=MT) as accp,
            tc.tile_pool(name="psp", bufs=2, space="PSUM") as psp,
        ):
            et_sb, eb_sb = [], []
            for k in range(KT):
                tk = etp.tile([P, B], fp16, tag=f"et{k}")
                nc.sync.dma_start(out=tk, in_=et[k * P:(k + 1) * P, :])
                et_sb.append(tk)
                bk = ebp.tile([P, ROWS], fp16, tag=f"eb{k}")
                nc.sync.dma_start(out=bk, in_=eblk[k * P:(k + 1) * P, :])
                eb_sb.append(bk)
            penl_sb = etp.tile([P, ROWS], fp16, tag="penl")
            nc.sync.dma_start(out=penl_sb, in_=penl[:, :])
            penr_sb = etp.tile([P, B], fp16, tag="penr")
            nc.sync.dma_start(out=penr_sb, in_=penr[:, :])

            for r in range(reps):
              for t in range(MT):
                ms = slice(t * P, (t + 1) * P)
                hp_acc = accp.tile([P, NG], f32, tag="hp")
                hn_acc = accp.tile([P, NG], f32, tag="hn")
                hp_f = accp.tile([P, 1], f32, tag="hpf")
                hn_f = accp.tile([P, 1], f32, tag="hnf")
                for g in range(NG):
                    ps = psp.tile([P, GW], f32)
                    # penalty + column-term injection (start=True clears bank)
                    for j in range(GW // NW):
                        cs = slice(g * GW + j * NW, g * GW + (j + 1) * NW)
                        nc.tensor.matmul(
                            ps[:, j * NW:(j + 1) * NW],
                            penl_sb[:, ms], penr_sb[:, cs],
                            start=True, stop=False,
                        )
                    # gram accumulation: w = colterm + PEN*same - 2 e_m.e_n
                    for k in range(KT):
                        for j in range(GW // NW):
                            cs = slice(g * GW + j * NW, g * GW + (j + 1) * NW)
                            nc.tensor.matmul(
                                ps[:, j * NW:(j + 1) * NW],
                                eb_sb[k][:, ms], et_sb[k][:, cs],
                                start=False, stop=(k == KT - 1),
                            )
                    nc.vector.tensor_reduce(
                        hp_acc[:, g:g + 1], ps, mybir.AxisListType.X,
                        mybir.AluOpType.max,
                    )
                    nc.vector.tensor_reduce(
                        hn_acc[:, g:g + 1], ps, mybir.AxisListType.X,
                        mybir.AluOpType.min,
                    )
                nc.vector.tensor_reduce(
                    hp_f, hp_acc, mybir.AxisListType.X, mybir.AluOpType.max
                )
                nc.vector.tensor_reduce(
                    hn_f, hn_acc, mybir.AxisListType.X, mybir.AluOpType.min
                )
                nc.sync.dma_start(out=outd[r, 0, t, :], in_=hp_f)
                nc.sync.dma_start(out=outd[r, 1, t, :], in_=hn_f)
    nc.compile()
    return nc


def _get_nc(reps=1):
    if reps not in _nc_cache:
        _nc_cache[reps] = _build(reps)
    return _nc_cache[reps]


def _prepare_inputs(embeddings, labels):
    Ef = np.ascontiguousarray(np.asarray(embeddings, dtype=np.float32))
    lab = np.asarray(labels).astype(np.int64)
    perm = np.argsort(lab, kind="stable")
    Ef = Ef[perm]
    labp = lab[perm]

    sq = np.sum(Ef * Ef, axis=1, dtype=np.float32)          # [B]
    s = np.sum(Ef, axis=1, dtype=np.float32)                # [B]
    et16 = np.ascontiguousarray(
        (Ef * np.float32(np.sqrt(2.0))).T.astype(np.float16))   # [D, B]

    colterm = (sq - 2.0 * EPS * s).astype(np.float32)
    colhi = colterm.astype(np.float16)
    collo = (colterm - colhi.astype(np.float32)).astype(np.float16)
    rowterm = (sq + 2.0 * EPS * s + D * EPS * EPS).astype(np.float32)

    # global class segments in sorted order
    # seg_start[q], seg_end[q] for each distinct label value
    uniq, first = np.unique(labp, return_index=True)
    bounds = np.r_[first, B]
    seg_of_col = np.searchsorted(labp, labp, side="left")   # start idx per col
    pen_val = np.float16(128.0)

    in_maps = []
    for c in range(N_CORES):
        r0, r1 = c * ROWS, (c + 1) * ROWS
        # distinct classes among this core's rows
        cls_ids = np.unique(labp[r0:r1])
        assert len(cls_ids) <= NCLASS_MAX, len(cls_ids)
        dim_of = {q: i for i, q in enumerate(cls_ids)}

        penl_a = np.zeros((P, ROWS), dtype=np.float16)
        for i in range(ROWS):
            penl_a[dim_of[labp[r0 + i]], i] = pen_val
        penl_a[P - 2, :] = np.float16(1.0)
        penl_a[P - 1, :] = np.float16(1.0)

        penr_a = np.zeros((P, B), dtype=np.float16)
        for q in cls_ids:
            qi = np.searchsorted(uniq, q)
            a, b = bounds[qi], bounds[qi + 1]
            penr_a[dim_of[q], a:b] = pen_val
        penr_a[P - 2, :] = colhi
        penr_a[P - 1, :] = collo

        in_maps.append({
            "et": et16,
            "eblk": np.ascontiguousarray(-et16[:, r0:r1]),
            "penl": penl_a,
            "penr": penr_a,
        })
    return in_maps, labp, rowterm


def _postprocess(results, labp, rowterm):
    hp_raw = np.concatenate([r["out"][0][0].reshape(-1) for r in results])  # [B]
    hn_raw = np.concatenate([r["out"][0][1].reshape(-1) for r in results])  # [B]
    hp2 = hp_raw - np.float32(PEN) + rowterm
    hn2 = hn_raw + rowterm
    hp = np.sqrt(np.maximum(hp2, 0.0, dtype=np.float32))
    hn = np.sqrt(np.maximum(hn2, 0.0, dtype=np.float32))

    cnt_lab = np.bincount(labp, minlength=1)
    n_same = cnt_lab[labp]
    valid = (n_same > 1) & (n_same < B)
    per = np.where(valid, np.maximum(hp - hn + np.float32(MARGIN), 0.0), 0.0)
    cnt = np.float32(valid.sum())
    if cnt > 0:
        loss = np.float32(per.sum(dtype=np.float32) / max(cnt, np.float32(1.0)))
    else:
        loss = np.float32(0.0)
    return np.asarray(loss, dtype=np.float32)


def _run(in_maps, reps=1, **kw):
    nc = _get_nc(reps)
    return run_bass_kernel_spmd(nc, in_maps, core_ids=list(range(N_CORES)), **kw)


def kernel(embeddings, labels):
    in_maps, labp, rowterm = _prepare_inputs(embeddings, labels)
    res = _run(in_maps)
    return _postprocess(res.results, labp, rowterm)


# revision 25
# speedup vs baseline: 1.1086x; 1.1086x over previous
"""BatchHardTripletLoss on 8 Trainium2 NeuronCores.

Strategy (data-parallel over anchor rows, samples pre-sorted by label):
  - host sorts samples by label (loss is permutation-invariant); core c owns
    anchor rows [c*512, (c+1)*512).
  - sorting clusters each core's 512 rows into <=126 distinct label classes,
    so the same-label penalty matrix PEN * 1[label_m == label_n] restricted to
    a core's rows is RANK <= 126: PEN * onehot_m . onehot_n over class dims.
    It is injected by ONE extra K=128 matmul per output tile (values 128*128 =
    PEN exactly in fp16). Two spare class dims carry the column terms
    (|e_n|^2 - 2*eps*sum(e_n), split into fp16 hi+lo rows against ones).
  - per (m,n) tile: pen/colterm matmul (start=True) + 4 gram matmuls
    accumulate  w = colterm[n] + PEN*same - 2 e_m.e_n  directly in PSUM.
  - DVE mines row-max (hardest positive + PEN) and row-min (hardest negative)
    straight out of PSUM; row-constant terms are applied on the host.
  - host: subtract PEN, add row terms, sqrt, validity via label bincount, mean.
"""

import dataclasses

import numpy as np

import concourse.bacc as bacc
import concourse.mybir as mybir
from concourse.bass_utils import run_bass_kernel_spmd
from concourse.tile import TileContext
from concourse import dve_ops as _dve_ops
from concourse.dve_spec import (
    AluOp, C0, C1, C2, Idx, Spec, Src0, lower, ne, scan, select,
)
from concourse.dve_uop import DveOpSpec


def _register_dual_op():
    """One DVE pass over a [P, N] tile producing BOTH reductions:
    accum_out = max(seed_s1, body) where body = w except at the last index,
    out[:, N-1] = running min (seeded +FLT_MAX via imm2) = total row min.
    The true max of the last column is restored by a 1-element fixup op.
    """
    name = "ANT_MINMAX_DUAL"
    for op in _dve_ops.OPS:
        if op.name == name:
            return op
    spec = Spec(
        body=select(ne(Idx, C0), Src0, scan(AluOp.MIN, Src0, init=C2)),
        accum=AluOp.MAX,
        accum_init=C1,
        reference=lambda in0, s0, s1, imm2: np.where(
            np.arange(in0.shape[-1]) != s0,
            in0,
            np.minimum.accumulate(np.minimum(in0, imm2), axis=-1),
        ),
    )
    op = _dve_ops.DveOp(name, spec, subdim=False, uops_sha={})
    _dve_ops.OPS.append(op)
    opcode = _dve_ops._CUSTOM_DVE_ROW_BASE + len(_dve_ops.OPS) - 1
    assert opcode < 0x20
    _dve_ops._SUB_OPCODE_FOR_NAME[name] = opcode
    _dve_ops.CUSTOM_DVE_SPECS[name] = spec
    shas = {}
    for ver in ("v3", "v4"):
        s = DveOpSpec(name=name, opcode=opcode, uops=lower(spec, ver=ver),
                      rd1_en=False)
        shas[ver] = s.sha(ver)
    op = dataclasses.replace(op, uops_sha=shas)
    _dve_ops.OPS[-1] = op
    return op


DUAL_OP = _register_dual_op()

B = 4096          # batch (anchors)
D = 512           # embedding dim
N_CORES = 8
ROWS = B // N_CORES      # 512 anchor rows per core
P = 128                  # partitions
MT = ROWS // P           # 4 m-tiles per core
NW = 512                 # psum bank width (fp32)
GW = 2048                # column group width (4 banks)
NG = B // GW             # 2 column groups
KT = D // P              # 4 contraction tiles

PEN = 16384.0            # same-label penalty; must exceed max d2 (~2.7k here)
NCLASS_MAX = 126         # class dims per core (2 reserved for colterm hi/lo)
MARGIN = 0.5
EPS = 1e-6

_nc_cache = {}


def _build(reps=1):
    nc = bacc.Bacc("TRN2", target_bir_lowering=False)
    fp16 = mybir.dt.float16
    f32 = mybir.dt.float32

    et = nc.dram_tensor("et", [D, B], fp16, kind="ExternalInput")
    eblk = nc.dram_tensor("eblk", [D, ROWS], fp16, kind="ExternalInput")
    penl = nc.dram_tensor("penl", [P, ROWS], fp16, kind="ExternalInput")
    penr = nc.dram_tensor("penr", [P, B], fp16, kind="ExternalInput")
    outd = nc.dram_tensor("out", [reps, 2, MT, P], f32, kind="ExternalOutput")

    NEG_INIT = -3.0e38
    SCAN_INIT = 3.0e38
    with TileContext(nc) as tc:
        with (
            tc.tile_pool(name="etp", bufs=1) as etp,
            tc.tile_pool(name="ebp", bufs=1) as ebp,
            tc.tile_pool(name="wp", bufs=2) as wp,
            tc.tile_pool(name="accp", bufs=MT) as accp,
            tc.tile_pool(name="psp", bufs=2, space="PSUM") as psp,
        ):
            # --- PE warmup: dense tiny matmuls while input DMAs run -------
            warm = etp.tile([P, 64], fp16, tag="warm")
            nc.gpsimd.memset(warm, 0.0)
            wps = psp.tile([P, GW], f32, tag="ps", name="wps")
            for _ in range(36):
                nc.tensor.matmul(wps[:64, 0:64], warm[:, 0:64], warm[:, 0:64],
                                 start=True, stop=True)

            # --- input DMAs, critical-path first --------------------------
            penl_sb = etp.tile([P, ROWS], fp16, tag="penl")
            nc.sync.dma_start(out=penl_sb, in_=penl[:, :])
            penr_sb = etp.tile([P, B], fp16, tag="penr")
            nc.sync.dma_start(out=penr_sb[:, 0:GW], in_=penr[:, 0:GW])
            et_sb, eb_sb = [], []
            for k in range(KT):
                et_sb.append(etp.tile([P, B], fp16, tag=f"et{k}", name=f"et{k}"))
                eb_sb.append(ebp.tile([P, ROWS], fp16, tag=f"eb{k}", name=f"eb{k}"))
            nc.sync.dma_start(out=eb_sb[0], in_=eblk[0:P, :])
            nc.sync.dma_start(out=et_sb[0][:, 0:GW], in_=et[0:P, 0:GW])
            nc.sync.dma_start(out=penr_sb[:, GW:B], in_=penr[:, GW:B])
            nc.sync.dma_start(out=et_sb[0][:, GW:B], in_=et[0:P, GW:B])
            for k in range(1, KT):
                nc.sync.dma_start(out=eb_sb[k], in_=eblk[k * P:(k + 1) * P, :])
                nc.sync.dma_start(out=et_sb[k], in_=et[k * P:(k + 1) * P, :])

            for r in range(reps):
              for t in range(MT):
                ms = slice(t * P, (t + 1) * P)
                hp_acc = accp.tile([P, NG], f32, tag="hp")
                hn_acc = accp.tile([P, NG], f32, tag="hn")
                for g in range(NG):
                    ps = psp.tile([P, GW], f32, tag="ps", name="ps")
                    # penalty + column-term injection (start=True clears bank)
                    for j in range(GW // NW):
                        cs = slice(g * GW + j * NW, g * GW + (j + 1) * NW)
                        nc.tensor.matmul(
                            ps[:, j * NW:(j + 1) * NW],
                            penl_sb[:, ms], penr_sb[:, cs],
                            start=True, stop=False,
                        )
                    # gram accumulation: w = colterm + PEN*same - 2 e_m.e_n
                    for k in range(KT):
                        for j in range(GW // NW):
                            cs = slice(g * GW + j * NW, g * GW + (j + 1) * NW)
                            nc.tensor.matmul(
                                ps[:, j * NW:(j + 1) * NW],
                                eb_sb[k][:, ms], et_sb[k][:, cs],
                                start=False, stop=(k == KT - 1),
                            )
                    # fused mining: accum_out = row-max (chained via s1),
                    # scratch[:, -1] = row-min (scan), fixup col GW-1 into max
                    scratch = wp.tile([P, GW], f32, tag="scr")
                    nc.vector._custom_dve(
                        DUAL_OP,
                        out=scratch,
                        in0=ps,
                        s0=float(GW - 1),
                        s1=(NEG_INIT if g == 0 else hp_acc[:, g - 1:g]),
                        imm2=SCAN_INIT,
                        accum_out=hp_acc[:, g:g + 1],
                    )
                    # restore the excluded last column into the max
                    nc.vector.tensor_tensor(
                        hp_acc[:, g:g + 1], ps[:, GW - 1:GW],
                        hp_acc[:, g:g + 1], mybir.AluOpType.max,
                    )
                    # chain the min across groups
                    if g == 0:
                        nc.vector.tensor_copy(
                            hn_acc[:, 0:1], scratch[:, GW - 1:GW])
                    else:
                        nc.vector.tensor_tensor(
                            hn_acc[:, g:g + 1], scratch[:, GW - 1:GW],
                            hn_acc[:, g - 1:g], mybir.AluOpType.min,
                        )
                nc.sync.dma_start(out=outd[r, 0, t, :], in_=hp_acc[:, NG - 1:NG])
                nc.sync.dma_start(out=outd[r, 1, t, :], in_=hn_acc[:, NG - 1:NG])
    nc.compile()
    return nc


def _get_nc(reps=1):
    if reps not in _nc_cache:
        _nc_cache[reps] = _build(reps)
    return _nc_cache[reps]


def _prepare_inputs(embeddings, labels):
    Ef = np.ascontiguousarray(np.asarray(embeddings, dtype=np.float32))
    lab = np.asarray(labels).astype(np.int64)
    perm = np.argsort(lab, kind="stable")
    Ef = Ef[perm]
    labp = lab[perm]

    sq = np.sum(Ef * Ef, axis=1, dtype=np.float32)          # [B]
    s = np.sum(Ef, axis=1, dtype=np.float32)                # [B]
    et16 = np.ascontiguousarray(
        (Ef * np.float32(np.sqrt(2.0))).T.astype(np.float16))   # [D, B]

    colterm = (sq - 2.0 * EPS * s).astype(np.float32)
    colhi = colterm.astype(np.float16)
    collo = (colterm - colhi.astype(np.float32)).astype(np.float16)
    rowterm = (sq + 2.0 * EPS * s + D * EPS * EPS).astype(np.float32)

    # global class segments in sorted order
    # seg_start[q], seg_end[q] for each distinct label value
    uniq, first = np.unique(labp, return_index=True)
    bounds = np.r_[first, B]
    seg_of_col = np.searchsorted(labp, labp, side="left")   # start idx per col
    pen_val = np.float16(128.0)

    in_maps = []
    for c in range(N_CORES):
        r0, r1 = c * ROWS, (c + 1) * ROWS
        # distinct classes among this core's rows
        cls_ids = np.unique(labp[r0:r1])
        assert len(cls_ids) <= NCLASS_MAX, len(cls_ids)
        dim_of = {q: i for i, q in enumerate(cls_ids)}

        penl_a = np.zeros((P, ROWS), dtype=np.float16)
        for i in range(ROWS):
            penl_a[dim_of[labp[r0 + i]], i] = pen_val
        penl_a[P - 2, :] = np.float16(1.0)
        penl_a[P - 1, :] = np.float16(1.0)

        penr_a = np.zeros((P, B), dtype=np.float16)
        for q in cls_ids:
            qi = np.searchsorted(uniq, q)
            a, b = bounds[qi], bounds[qi + 1]
            penr_a[dim_of[q], a:b] = pen_val
        penr_a[P - 2, :] = colhi
        penr_a[P - 1, :] = collo

        in_maps.append({
            "et": et16,
            "eblk": np.ascontiguousarray(-et16[:, r0:r1]),
            "penl": penl_a,
            "penr": penr_a,
        })
    return in_maps, labp, rowterm


def _postprocess(results, labp, rowterm):
    hp_raw = np.concatenate([r["out"][0][0].reshape(-1) for r in results])  # [B]
    hn_raw = np.concatenate([r["out"][0][1].reshape(-1) for r in results])  # [B]
    hp2 = hp_raw - np.float32(PEN) + rowterm
    hn2 = hn_raw + rowterm
    hp = np.sqrt(np.maximum(hp2, 0.0, dtype=np.float32))
    hn = np.sqrt(np.maximum(hn2, 0.0, dtype=np.float32))

    cnt_lab = np.bincount(labp, minlength=1)
    n_same = cnt_lab[labp]
    valid = (n_same > 1) & (n_same < B)
    per = np.where(valid, np.maximum(hp - hn + np.float32(MARGIN), 0.0), 0.0)
    cnt = np.float32(valid.sum())
    if cnt > 0:
        loss = np.float32(per.sum(dtype=np.float32) / max(cnt, np.float32(1.0)))
    else:
        loss = np.float32(0.0)
    return np.asarray(loss, dtype=np.float32)


def _run(in_maps, reps=1, **kw):
    nc = _get_nc(reps)
    return run_bass_kernel_spmd(nc, in_maps, core_ids=list(range(N_CORES)), **kw)


def kernel(embeddings, labels):
    in_maps, labp, rowterm = _prepare_inputs(embeddings, labels)
    res = _run(in_maps)
    return _postprocess(res.results, labp, rowterm)
